# revision 10
# baseline (speedup 1.0000x reference)
"""Trainium2 Bass kernel for nn_MultiHeadAttention_9491877724818.

Math (per batch b, head h), reformulated from the reference:
    q = Wq_h @ x_b + bq          (128, T)
    k = Wk_h @ x_b + bk          (128, T)
    eT[s,t] = (k.T @ q)[s,t]     == energy[t,s]; softmax over s (partition dim)
    expET = exp(eT)              (no max subtraction: |logit| <= ~70, fp32-safe)
    Z[t] = sum_s expET[s,t]      (PE ones-matmul -> broadcast across partitions)
Key algebraic folding: heads only enter the output through W1 (DFC1=128 rows),
so the huge Wv (C x C) conv and o = v @ attn collapse into 128-channel
products:
    vW1T[s,j]  = (x_b.T @ (W1 @ Wv_h).T)[s,j]          (T, 128)
    oW1raw[j,t]= sum_s vW1T[s,j] expET[s,t]            (128, T)
    fc1[j,t]   = relu(gamma_h * oW1raw[j,t]/Z[t] + xW1[b][j,t] + b1eff_h[j])
    out2[d,t]  = relu(W2 @ fc1 + b2)
    final[b, 8*d + h, t] = out2[d,t] + x[b, 8*d + h, t]

Sharding: data parallel - core i computes batch b=i entirely (all 8 heads).

v2 changes vs the 181us baseline:
  - Startup: x fp16 is loaded as 8 per-ci tiles and phase A (xW1 + head-0
    QK convs) is emitted interleaved per ci, so the PE starts ~2us in and
    trickles at DMA pace instead of idling 12.5us for the full 2MB load.
  - vW1T runs in fp8e4 DoubleRow (K=256 per instruction): 128 bf16 matmuls
    -> 64 DR matmuls. Inputs x(*4) and W1Wv(*32) are pre-scaled on host
    (e4m3 subnormal avoidance); the 1/128 descale is folded into gamma.
    Numpy study: rel err 0.0065 vs 2e-2 budget.
  - exp is fused 2-wide: eT matmuls write [P,2,512] psum quads, one ACT
    instruction exps both banks (ACT per-op overhead ~450ns dominated the
    old 8-op chain; ACT drops ~40us and stops gating the PE).
  - q/k psum->sbuf bias copies moved DVE->ACT (activation Copy with bias),
    residual add moved DVE->GPSIMD: DVE drops from ~138us to ~95us.
"""

import numpy as np

B, C, T, H, P = 8, 1024, 1024, 8, 128
CT = C // P      # 8 contraction k-tiles over channels
CP = CT // 2     # 4 ci-pairs for DoubleRow
ST = T // P      # 8 s-tiles (softmax/partition dim)
NT = 2           # t-chunks per row
TCW = T // NT    # 512 = matmul moving free dim

USE_DR_VW1T = True   # fp8 DoubleRow for the vW1T matmuls
SX, SW = 4.0, 32.0   # host pre-scales for x / W1Wv fp8 casts

_module_cache = {}


def _build_module(key=0):
    from contextlib import ExitStack

    import concourse.bacc as bacc
    import concourse.bass as bass
    import concourse.mybir as mybir
    import concourse.tile as tile

    f32 = mybir.dt.float32
    ldt = mybir.dt.float16    # logit path (QK convs, eT)
    mdt = mybir.dt.bfloat16   # post-softmax path
    vdt = mybir.dt.float8e4   # vW1T DoubleRow path
    AF = mybir.ActivationFunctionType
    ALU = mybir.AluOpType
    DR = mybir.MatmulPerfMode.DoubleRow

    nc = bacc.Bacc(trn_type="TRN2", name="mha_dp")

    # f32 x always present (residual source)
    x_d = nc.dram_tensor("x", (C, T), f32, kind="ExternalInput")
    xl_d = nc.dram_tensor("x_f16", (C, T), ldt, kind="ExternalInput")
    wqk_d = nc.dram_tensor("wqk", (H, P, CT, 256), ldt, kind="ExternalInput")
    if USE_DR_VW1T:
        xv8_d = nc.dram_tensor("xv8", (CP, P, 2, T), vdt, kind="ExternalInput")
        w1wv8_d = nc.dram_tensor("w1wv8", (CP, P, 2, H * P), vdt, kind="ExternalInput")
    else:
        w1wv_d = nc.dram_tensor("w1wv", (P, CT, H * P), ldt, kind="ExternalInput")
    w1t_d = nc.dram_tensor("w1t", (P, CT, P), ldt, kind="ExternalInput")
    w2t_d = nc.dram_tensor("w2t", (P, P), mdt, kind="ExternalInput")
    ones_d = nc.dram_tensor("ones", (P, P), mdt, kind="ExternalInput")
    bqk_d = nc.dram_tensor("bqk", (H, 2, P), f32, kind="ExternalInput")
    b1e_d = nc.dram_tensor("b1e", (H, P), f32, kind="ExternalInput")
    b2_d = nc.dram_tensor("b2", (P,), f32, kind="ExternalInput")
    gam_d = nc.dram_tensor("gam", (H,), f32, kind="ExternalInput")
    out_d = nc.dram_tensor("out", (C, T), f32, kind="ExternalOutput")

    def mm(ps, lhsT, rhs, start, stop, **kw):
        nc.tensor.matmul(ps, lhsT, rhs, start=start, stop=stop, **kw)

    with tile.TileContext(nc) as tc, ExitStack() as ctx:
        consts = ctx.enter_context(tc.tile_pool(name="consts", bufs=1))
        # PSUM budget (8 banks): eq 2x2 + qk 2 + oo 1 + zf 1 = 8
        pse = ctx.enter_context(tc.tile_pool(name="pse", bufs=2, space="PSUM"))
        psB = ctx.enter_context(tc.tile_pool(name="psB", bufs=1, space="PSUM"))

        wqkp = ctx.enter_context(tc.tile_pool(name="wqkp", bufs=2))
        qkp = ctx.enter_context(tc.tile_pool(name="qkp", bufs=2))
        expp = ctx.enter_context(tc.tile_pool(name="expp", bufs=3))
        hbuf = ctx.enter_context(tc.tile_pool(name="hbuf", bufs=2))
        outp = ctx.enter_context(tc.tile_pool(name="outp", bufs=2))

        # ------------- DMA priority order: first-compute tensors first.
        # HW-dynamic DMA only starts ~8.5us in (NEFF preamble) and ramps
        # slowly, so the bytes ahead of xl[0] directly delay the first matmul.
        w1t_sb = consts.tile([P, CT, P], ldt, name="w1t_sb")
        nc.sync.dma_start(out=w1t_sb, in_=w1t_d[:])

        xl_sb = [consts.tile([P, T], ldt, name=f"xl{ci}_sb") for ci in range(CT)]
        nc.sync.dma_start(out=xl_sb[0], in_=xl_d[0:P, :])

        head_state = {}
        chunk_state = {}
        prefetched = {}

        def prefetch_head(h):
            wqk_sb = wqkp.tile([P, CT, 256], ldt, name="wqk_sb", tag="wqk")
            nc.sync.dma_start(out=wqk_sb, in_=wqk_d[h])
            bq_sb = wqkp.tile([P, 1], f32, name="bq_sb", tag="bq")
            nc.sync.dma_start(out=bq_sb, in_=bqk_d[h, 0, :])
            bk_sb = wqkp.tile([P, 1], f32, name="bk_sb", tag="bk")
            nc.sync.dma_start(out=bk_sb, in_=bqk_d[h, 1, :])
            b1e_sb = wqkp.tile([P, 1], f32, name="b1e_sb", tag="b1e")
            nc.sync.dma_start(out=b1e_sb, in_=b1e_d[h, :])
            gam_sb = wqkp.tile([P, 1], f32, name="gam_sb", tag="gam")
            gam_ap = gam_d[h : h + 1]
            nc.gpsimd.dma_start(
                out=gam_sb,
                in_=bass.AP(tensor=gam_ap.tensor, offset=gam_ap.offset, ap=[[0, P], [1, 1]]),
            )
            prefetched[h] = dict(wqk=wqk_sb, bq=bq_sb, bk=bk_sb, b1e=b1e_sb, gam=gam_sb)

        def prefetch_xres(h):
            # residual rows h, h+8, ..., h+8*127 of x; sync queue so its
            # position in the load order is explicit
            xres_sb = wqkp.tile([P, T], f32, name="xres_sb", tag="xres")
            x_all = x_d[:]
            nc.sync.dma_start(
                out=xres_sb,
                in_=bass.AP(tensor=x_all.tensor, offset=h * T, ap=[[H * T, P], [1, T]]),
            )
            prefetched[h]["xres"] = xres_sb

        prefetch_head(0)
        for ci in range(1, CT):
            nc.sync.dma_start(out=xl_sb[ci], in_=xl_d[ci * P : (ci + 1) * P, :])

        if USE_DR_VW1T:
            xv8_sb, w1wv8_sb = [], []
            for j in range(CP):
                tw = consts.tile([P, 2, H * P], vdt, name=f"w1wv8{j}_sb")
                nc.sync.dma_start(out=tw, in_=w1wv8_d[j])
                w1wv8_sb.append(tw)
                tx = consts.tile([P, 2, T], vdt, name=f"xv8{j}_sb")
                nc.sync.dma_start(out=tx, in_=xv8_d[j])
                xv8_sb.append(tx)
        else:
            w1wv_sb = consts.tile([P, CT, H * P], ldt, name="w1wv_sb")
            for ci in range(CT):
                nc.sync.dma_start(out=w1wv_sb[:, ci, :], in_=w1wv_d[:, ci, :])

        ones_sb = consts.tile([P, P], mdt, name="ones_sb")
        nc.sync.dma_start(out=ones_sb, in_=ones_d[:])
        b2_sb = consts.tile([P, 1], f32, name="b2_sb")
        nc.sync.dma_start(out=b2_sb, in_=b2_d[:])
        w2t_sb = consts.tile([P, P], mdt, name="w2t_sb")
        nc.sync.dma_start(out=w2t_sb, in_=w2t_d[:])
        prefetch_xres(0)

        xw1_sb = consts.tile([P, NT, TCW], f32, name="xw1_sb")
        vw1t_sb = consts.tile([P, ST, 2, TCW], mdt, name="vw1t_sb")

        def emit_head_state(h):
            pf = prefetched.pop(h)
            q_sb = qkp.tile([P, NT, TCW], ldt, name="q_sb", tag="q")
            k_sb = qkp.tile([P, T], ldt, name="k_sb", tag="k")
            out_sb = outp.tile([P, T], f32, name="out_sb", tag="ob")
            return dict(
                q=q_sb, k=k_sb, gam=pf["gam"], b1e=pf["b1e"], bq=pf["bq"],
                bk=pf["bk"], wqk=pf["wqk"], xres=pf["xres"], out=out_sb
            )

        def finish_qk(hs, wqk_sb, ps_q, ps_k0, ps_k1):
            # psum -> sbuf with bias, on ACT (DVE is the loaded engine)
            nc.scalar.activation(out=hs["k"][:, 0:TCW], in_=ps_k0, func=AF.Identity, bias=hs["bk"])
            nc.scalar.activation(out=hs["k"][:, TCW:T], in_=ps_k1, func=AF.Identity, bias=hs["bk"])
            nc.scalar.activation(out=hs["q"][:, :, :], in_=ps_q, func=AF.Identity, bias=hs["bq"])

        def emit_head_setup(h):
            # heads >= 1: xl fully resident; weights prefetched a head ago
            if h + 1 < H:
                prefetch_head(h + 1)
                prefetch_xres(h + 1)
            hs = emit_head_state(h)
            wqk_sb = hs["wqk"]
            ps_k0 = psB.tile([P, TCW], f32, name="ps_k0", tag="qk", bufs=2)
            ps_k1 = psB.tile([P, TCW], f32, name="ps_k1", tag="qk", bufs=2)
            ps_q = pse.tile([P, NT, TCW], f32, name="ps_q", tag="eq")
            for ci in range(CT):
                mm(ps_k0, wqk_sb[:, ci, P : 2 * P], xl_sb[ci][:, 0:TCW], ci == 0, ci == CT - 1)
                mm(ps_k1, wqk_sb[:, ci, P : 2 * P], xl_sb[ci][:, TCW:T], ci == 0, ci == CT - 1)
            for ci in range(CT):
                mm(ps_q[:, 0, :], wqk_sb[:, ci, 0:P], xl_sb[ci][:, 0:TCW], ci == 0, ci == CT - 1)
                mm(ps_q[:, 1, :], wqk_sb[:, ci, 0:P], xl_sb[ci][:, TCW:T], ci == 0, ci == CT - 1)
            finish_qk(hs, wqk_sb, ps_q, ps_k0, ps_k1)
            head_state[h] = hs

        def emit_phase_a_and_head0():
            # xW1 + head-0 QK convs interleaved per ci so compute trickles
            # behind the per-ci x DMAs; then vW1T (fp8 DR).
            prefetch_head(1)
            prefetch_xres(1)
            hs = emit_head_state(0)
            wqk0_sb = hs["wqk"]
            quadX = pse.tile([P, NT, TCW], f32, name="quadX", tag="eq")
            ps_q = pse.tile([P, NT, TCW], f32, name="ps_q", tag="eq")
            ps_k0 = psB.tile([P, TCW], f32, name="ps_k0", tag="qk", bufs=2)
            ps_k1 = psB.tile([P, TCW], f32, name="ps_k1", tag="qk", bufs=2)
            for ci in range(CT):
                st, sp = ci == 0, ci == CT - 1
                mm(quadX[:, 0, :], w1t_sb[:, ci, :], xl_sb[ci][:, 0:TCW], st, sp)
                mm(quadX[:, 1, :], w1t_sb[:, ci, :], xl_sb[ci][:, TCW:T], st, sp)
                mm(ps_k0, wqk0_sb[:, ci, P : 2 * P], xl_sb[ci][:, 0:TCW], st, sp)
                mm(ps_k1, wqk0_sb[:, ci, P : 2 * P], xl_sb[ci][:, TCW:T], st, sp)
                mm(ps_q[:, 0, :], wqk0_sb[:, ci, 0:P], xl_sb[ci][:, 0:TCW], st, sp)
                mm(ps_q[:, 1, :], wqk0_sb[:, ci, 0:P], xl_sb[ci][:, TCW:T], st, sp)
            finish_qk(hs, wqk0_sb, ps_q, ps_k0, ps_k1)
            nc.vector.tensor_copy(out=xw1_sb[:, :, :], in_=quadX[:, :, :])
            head_state[0] = hs

            # vW1T for all heads: DR fp8, K=256 per matmul over ci-pairs
            for si in range(ST):
                psv = pse.tile([P, NT, TCW], f32, name="psv", tag="eq")
                for jh in range(2):
                    for jp in range(CP):
                        if USE_DR_VW1T:
                            mm(
                                psv[:, jh, :],
                                xv8_sb[jp][:, :, si * P : (si + 1) * P],
                                w1wv8_sb[jp][:, :, jh * TCW : (jh + 1) * TCW],
                                jp == 0,
                                jp == CP - 1,
                                perf_mode=DR,
                            )
                        else:
                            for ci in (2 * jp, 2 * jp + 1):
                                mm(
                                    psv[:, jh, :],
                                    xl_sb[ci][:, si * P : (si + 1) * P],
                                    w1wv_sb[:, ci, jh * TCW : (jh + 1) * TCW],
                                    ci == 0,
                                    ci == CT - 1,
                                )
                nc.vector.tensor_copy(out=vw1t_sb[:, si, :, :], in_=psv[:, :, :])

        def emit_s1_half(c, first):
            h, t2 = c
            hs = head_state[h]
            if first:
                et_sb = expp.tile([P, ST, TCW], mdt, name="et_sb", tag="exp")
                chunk_state[c] = dict(et=et_sb)
            else:
                et_sb = chunk_state[c]["et"]
            base = 0 if first else ST // 2
            for pr in range(2):
                si0 = base + 2 * pr
                pe = pse.tile([P, 2, TCW], f32, name="pe", tag="eq")
                for i in range(2):
                    si = si0 + i
                    mm(pe[:, i, :], hs["k"][:, si * P : (si + 1) * P], hs["q"][:, t2, :], True, True)
                nc.scalar.activation(out=et_sb[:, si0 : si0 + 2, :], in_=pe[:, :, :], func=AF.Exp)

        def emit_s2_mm(c):
            h, t2 = c
            cs = chunk_state[c]
            et_sb = cs["et"]
            ps_o = psB.tile([P, TCW], f32, name="ps_o", tag="oo")
            for si in range(ST):
                mm(
                    ps_o,
                    vw1t_sb[:, si, h // 4, (h % 4) * P : (h % 4 + 1) * P],
                    et_sb[:, si, :],
                    si == 0,
                    si == ST - 1,
                )
            cs["ps_o"] = ps_o

        def emit_s2_s3(c):
            h, t2 = c
            hs = head_state[h]
            cs = chunk_state[c]
            et_sb = cs["et"]
            ps_o = cs["ps_o"]
            # Z: tree-sum the 8 s-tiles on DVE, then one ones-matmul for the
            # partition reduction + broadcast.
            r1 = hbuf.tile([P, 4, TCW], mdt, name="r1", tag="r1")
            nc.vector.tensor_add(r1, et_sb[:, 0:4, :], et_sb[:, 4:8, :])
            r2 = hbuf.tile([P, 2, TCW], mdt, name="r2", tag="r2")
            nc.vector.tensor_add(r2, r1[:, 0:2, :], r1[:, 2:4, :])
            etsum = hbuf.tile([P, TCW], mdt, name="etsum", tag="etsum")
            nc.vector.tensor_add(etsum, r2[:, 0, :], r2[:, 1, :])
            ps_z = psB.tile([P, TCW], f32, name="ps_z", tag="zf")
            mm(ps_z, ones_sb, etsum, True, True)
            # fc1 = relu(gamma * oW1/Z + xW1 + b1eff), all on DVE
            izg = hbuf.tile([P, TCW], f32, name="izg", tag="izg")
            nc.vector.reciprocal_approx_fast(out=izg, in_=ps_z)
            t1 = hbuf.tile([P, TCW], f32, name="t1", tag="t1")
            nc.vector.scalar_tensor_tensor(
                out=t1, in0=ps_o, scalar=hs["gam"], in1=izg, op0=ALU.mult, op1=ALU.mult
            )
            t2t = hbuf.tile([P, TCW], f32, name="t2t", tag="t2t")
            nc.vector.scalar_tensor_tensor(
                out=t2t, in0=t1, scalar=hs["b1e"], in1=xw1_sb[:, t2, :], op0=ALU.add, op1=ALU.add
            )
            fc1 = hbuf.tile([P, TCW], mdt, name="fc1", tag="fc1")
            nc.vector.tensor_scalar_max(out=fc1, in0=t2t, scalar1=0.0)
            cs["fc1"] = fc1

        def emit_s4_s5(c, split=1):
            # split>1: pipeline the tail in split sub-slices (drain latency);
            # the final chunks also use DVE for the residual add (GPSIMD's
            # tensor_add is ~1.8us and would sit on the critical tail).
            h, t2 = c
            hs = head_state[h]
            cs = chunk_state[c]
            w = TCW // split
            for u in range(split):
                lo = t2 * TCW + u * w
                tsl = slice(lo, lo + w)
                ps_f = psB.tile([P, TCW], f32, name="ps_f", tag="zf")
                mm(ps_f[:, 0:w], w2t_sb[:], cs["fc1"][:, u * w : (u + 1) * w], True, True)
                ot = hbuf.tile([P, TCW], f32, name="ot", tag="ot")
                nc.scalar.activation(out=ot[:, 0:w], in_=ps_f[:, 0:w], func=AF.Relu, bias=b2_sb)
                eng = nc.vector if split > 1 else nc.gpsimd
                eng.tensor_add(hs["out"][:, tsl], ot[:, 0:w], hs["xres"][:, tsl])
                out_all = out_d[:]
                nc.sync.dma_start(
                    out=bass.AP(
                        tensor=out_all.tensor,
                        offset=h * T + lo,
                        ap=[[H * T, P], [1, w]],
                    ),
                    in_=hs["out"][:, tsl],
                )

        emit_phase_a_and_head0()

        chunks = [(h, t2) for h in range(H) for t2 in range(NT)]
        for i, c in enumerate(chunks):
            if c[1] == 0 and c[0] >= 1:
                emit_head_setup(c[0])
            # interleave: first half of this chunk's eT, then the previous
            # chunk's oW1 matmuls (gives the exp chain time to recycle the
            # eT psum slots), then the second half, then the rest.
            emit_s1_half(c, True)
            if i >= 1:
                emit_s2_mm(chunks[i - 1])
            emit_s1_half(c, False)
            if i >= 1:
                emit_s2_s3(chunks[i - 1])
            if i >= 2:
                emit_s4_s5(chunks[i - 2])
        emit_s2_mm(chunks[-1])
        emit_s4_s5(chunks[-2], split=2)
        emit_s2_s3(chunks[-1])
        emit_s4_s5(chunks[-1], split=4)

    nc.compile()
    return nc


def _prepare_inputs(inputs):
    import ml_dtypes

    E4 = ml_dtypes.float8_e4m3

    x = np.ascontiguousarray(np.asarray(inputs["x"], dtype=np.float32))
    Wq = np.asarray(inputs["Wq"], dtype=np.float32)
    bq = np.asarray(inputs["bq"], dtype=np.float32)
    Wk = np.asarray(inputs["Wk"], dtype=np.float32)
    bk = np.asarray(inputs["bk"], dtype=np.float32)
    Wv = np.asarray(inputs["Wv"], dtype=np.float32)
    bv = np.asarray(inputs["bv"], dtype=np.float32)
    gamma = np.asarray(inputs["gamma"], dtype=np.float32)
    W1 = np.asarray(inputs["W1"], dtype=np.float32)
    b1 = np.asarray(inputs["b1"], dtype=np.float32)
    W2 = np.asarray(inputs["W2"], dtype=np.float32)
    b2 = np.asarray(inputs["b2"], dtype=np.float32)

    # wqk[h, cp, ci, 0:128] = Wq[h].T[ci*128+cp, :]; 128:256 for Wk
    wqk = np.empty((H, P, CT, 256), dtype=np.float32)
    for h in range(H):
        wqk[h, :, :, 0:P] = Wq[h].T.reshape(CT, P, P).transpose(1, 0, 2)
        wqk[h, :, :, P : 2 * P] = Wk[h].T.reshape(CT, P, P).transpose(1, 0, 2)

    # w1wvT[c, h*128+j] = (W1 @ Wv[h]).T[c, j]
    w1wvT = np.concatenate([(W1 @ Wv[h]).T for h in range(H)], axis=1)  # (C, H*128)

    w1t = np.ascontiguousarray(W1.T.reshape(CT, P, P).transpose(1, 0, 2))
    w2t = np.ascontiguousarray(W2.T)

    bqk = np.stack([bq, bk], axis=1)  # (H, 2, P)
    b1v = bv @ W1.T  # (H, P): b1v[h] = W1 @ bv[h]
    b1e = b1[None, :] + gamma[:, None] * b1v  # (H, P)

    def e4(a, scale):
        return np.ascontiguousarray(
            np.clip(a * scale, -240.0, 240.0).astype(E4)
        )

    shared = {
        "wqk": np.ascontiguousarray(wqk.astype(np.float16)),
        "w1t": np.ascontiguousarray(w1t.astype(np.float16)),
        "w2t": np.ascontiguousarray(w2t.astype(ml_dtypes.bfloat16)),
        "ones": np.ones((P, P), dtype=ml_dtypes.bfloat16),
        "bqk": np.ascontiguousarray(bqk),
        "b1e": np.ascontiguousarray(b1e),
        "b2": np.ascontiguousarray(b2),
    }
    if USE_DR_VW1T:
        # w1wv8[j, p, i, m] = e4m3(32 * w1wvT[(2j+i)*128+p, m])
        shared["w1wv8"] = e4(
            w1wvT.reshape(CP, 2, P, H * P).transpose(0, 2, 1, 3), SW
        )
        # descale 1/(SX*SW) folded into gamma
        shared["gam"] = np.ascontiguousarray(gamma / (SX * SW))
    else:
        w1wv = np.ascontiguousarray(w1wvT.reshape(CT, P, H * P).transpose(1, 0, 2))
        shared["w1wv"] = np.ascontiguousarray(w1wv.astype(np.float16))
        shared["gam"] = np.ascontiguousarray(gamma)

    in_maps = []
    for b in range(B):
        m = dict(shared)
        m["x"] = np.ascontiguousarray(x[b])
        m["x_f16"] = np.ascontiguousarray(x[b].astype(np.float16))
        if USE_DR_VW1T:
            m["xv8"] = e4(x[b].reshape(CP, 2, P, T).transpose(0, 2, 1, 3), SX)
        in_maps.append(m)
    return in_maps


def kernel(**inputs):
    from concourse.bass_utils import run_bass_kernel_spmd

    if 0 not in _module_cache:
        _module_cache[0] = _build_module()
    nc = _module_cache[0]

    in_maps = _prepare_inputs(inputs)
    res = run_bass_kernel_spmd(nc, in_maps, core_ids=list(range(B)))
    out = np.stack([res.results[b]["out"] for b in range(B)], axis=0)
    return out.astype(np.float32)
